# revision 52
# baseline (speedup 1.0000x reference)
"""Trainium2 Bass kernel for nn_CrossAttention_8435315769719.

CrossAttention block: LN(x), LN(context), 12-head query / single shared
KV head cross-attention, output projection, plus a parallel SwiGLU FF on
the normed x.

Sharding: the 4096 query tokens (4 batches x 1024) are split into 8
row-slices of 512 tokens; core c handles batch c//2, token rows
(c%2)*512..+512, with that batch's full context. KV projection is
recomputed per core (cheap); no collectives needed.

Precision plan (rel-err budget 2e-2, measured ~8e-3 in numpy sim):
- x/ctx stream in as bf16; LN in bf16, xn/cn transposed via DMA-transpose
  (XBAR, 2-byte) straight into [dim, tok] layout - no PE transposes.
- Q/KV/FF1/FF2 GEMMs in bf16 (1 cyc/row): these carry the error budget.
- Attention core in fp8e4m3 with DoubleRow perf mode (0.5 cyc/row):
  sim = kT(64-partition DR with zero subtile) x q, attn*V contracting
  2 j-tiles per DR matmul, and the Wo projection (outT fp8 x Wo fp8 DR).
- exp() split across engines: ACT runs real Exp; DVE/GpSimd run a
  Schraudolph-style base-2 exp writing uint8 bit patterns that bitcast
  to fp8e4m3 (k = x*8*log2(e)*2^-11 + 16.105, saturating u8 convert).
- Power-of-2 scales keep fp8 operands in range and fold away exactly:
  Wq' = gx*Wq/8*2^7 (bf16), Wkv' = gc*Wkv*2^4, exp(sim_raw*2^-11-3.5),
  ones-col 2.0 => outT = out*2^3, Wo' = Wo*2^5 (fp8), Wff2' = Wff2*2^8,
  y = psum*2^-8.
"""

import functools

import numpy as np
import ml_dtypes

import concourse.bass as bass
import concourse.tile as tile
from concourse import mybir
from concourse.bass_utils import run_bass_kernel_spmd

# ---------------------------------------------------------------- sizes
DIM = 768
HEADS = 12
DH = 64
FFI = 3072  # FF inner (per u/gate half)
B = 4
N = 1024
J = 2048  # context length
EPS = 1e-5
NCORES = 8
TPC = 512  # query tokens per core

KC = DIM // 128  # 6 contraction chunks
TT = TPC // 128  # 4 token tiles per core
JT = J // 128  # 16 context tiles
FH = FFI // 128  # 24 ff tiles per half

F32 = mybir.dt.float32
F32R = mybir.dt.float32r
BF16 = mybir.dt.bfloat16
FP8 = mybir.dt.float8e4
U8 = mybir.dt.uint8

SUB = mybir.AluOpType.subtract
MULT = mybir.AluOpType.mult
ADD = mybir.AluOpType.add
AF = mybir.ActivationFunctionType
DR = mybir.MatmulPerfMode.DoubleRow

# scales (see module docstring)
EXP_SCALE = 2.0**-11
EXP_BIAS = -3.5
ONES_COL = 2.0
Y_SCALE = 2.0**-8
# Schraudolph-8: k = round(x*SCH_A + SCH_B) as saturating uint8, bitcast e4m3
SCH_A = 8.0 * np.log2(np.e) * EXP_SCALE
SCH_B = 56.0 - 8.0 * 3.5 * np.log2(np.e) + 0.5

# exp engine schedule: per (head, jp) 96 tiles; ACT does most, DVE + GPS offload
EXP_ENGINES = ("a", "a", "d", "a", "a", "d", "a", "d")  # per jp within a head


# ------------------------------------------------- tile drain workaround
def _install_drain_patch():
    """walrus in this container rejects >1 sync-wait on the Tile tail
    Drain ("Too many sync wait commands"). Split the final global-clock
    waits onto individual SP nops instead."""
    import concourse.tile as _t

    if getattr(_t.TileContext, "_drain_patched", False):
        return

    def _patched(self, tick_clock, wait_clock):
        nc = self.nc
        drain_inst = nc.sync.drain()
        wait_clock.add_sem_waits(
            drain_inst.ins, _t.ScopedClock({None: tick_clock.global_clock})
        )
        si = drain_inst.ins.sync_info
        waits = list(si.on_wait) if si is not None else []
        if si is not None and len(waits) > 1:
            si.on_wait = []
            for w in waits:
                n = nc.sync.nop(nofuse=True, hint="drain_split")
                n.ins.sync_info = mybir.SyncInfo(on_wait=[w], on_update=[])
        nc.all_engine_barrier()
        assert self.sems is not None
        popped = nc._tile_sem_poison_stack.pop()
        assert popped is self._sem_poison
        nc.clear_and_free_semaphores(list(self.sems.allocated().values()))
        nc.all_engine_barrier()

    _t.TileContext._drain_and_barrier = _patched
    _t.TileContext._drain_patched = True


def _split_excess_waits(nc, max_waits=1):
    """walrus in this container rejects instructions carrying more than
    ~2 sync waits ("Too many sync wait commands"). Move the excess onto
    same-engine NOPs inserted immediately before the instruction."""
    for fn in nc.m.functions:
        for bb in fn.blocks:
            rebuilt = []
            changed = False
            for inst in bb.instructions:
                si = inst.sync_info
                waits = list(si.on_wait) if si is not None else []
                if len(waits) > max_waits:
                    changed = True
                    si.on_wait = waits[-max_waits:]
                    excess = waits[:-max_waits]
                    for i0 in range(0, len(excess), max_waits):
                        nop = mybir.InstNoOp(
                            name=nc.get_next_instruction_name(),
                            sync_info=mybir.SyncInfo(
                                on_wait=excess[i0 : i0 + max_waits], on_update=[]
                            ),
                            bass_nofuse=True,
                            engine=inst.engine,
                        )
                        nc.register_instruction(nop, overwrite=True)
                        rebuilt.append(nop)
                rebuilt.append(inst)
            if changed:
                bb.instructions = rebuilt


# ------------------------------------------------------------ LN helper
def _ln_stats(nc, pool, xt, eps_t):
    """bn_stats/bn_aggr mean+var over the 768-wide free dim (3x256
    subgroups), returns (mu, rs) [128,1] APs ((x-mu)*rs is the LN)."""
    xg = xt.rearrange("p (s d) -> p s d", d=256)
    nsub = xg.shape[1]
    stats = pool.tile([128, nsub, nc.vector.BN_STATS_DIM], F32, tag="bnst")
    for s in range(nsub):
        nc.vector.bn_stats(out=stats[:, s, :], in_=xg[:, s, :])
    mv = pool.tile([128, nc.vector.BN_AGGR_DIM], F32, tag="mv")
    nc.vector.bn_aggr(out=mv, in_=stats)
    std = pool.tile([128, 1], F32, tag="std")
    nc.scalar.activation(out=std, in_=mv[:, 1:2], func=AF.Sqrt, bias=eps_t, scale=1.0)
    rs = pool.tile([128, 1], F32, tag="rs")
    nc.vector.reciprocal(out=rs, in_=std)
    return mv[:, 0:1], rs


# -------------------------------------------------------- program build
def _emit(nc):
    x_d = nc.dram_tensor("x", [TPC, DIM], BF16, kind="ExternalInput").ap()
    ctx_d = nc.dram_tensor("ctx", [J, DIM], BF16, kind="ExternalInput").ap()
    wq_d = nc.dram_tensor("wq", [DIM, DIM], FP8, kind="ExternalInput").ap()
    wkv_d = nc.dram_tensor("wkv", [DIM, 2 * DH], BF16, kind="ExternalInput").ap()
    wo_d = nc.dram_tensor("wo", [DIM, DIM], FP8, kind="ExternalInput").ap()
    wff1_d = nc.dram_tensor("wff1", [DIM, 2 * FFI], BF16, kind="ExternalInput").ap()
    wff2_d = nc.dram_tensor("wff2", [FFI, DIM], BF16, kind="ExternalInput").ap()
    ident8_d = nc.dram_tensor("ident8", [128, 128], FP8, kind="ExternalInput").ap()
    e2_d = nc.dram_tensor("e2", [33, 128], F32R, kind="ExternalInput").ap()
    y_d = nc.dram_tensor("y", [TPC, DIM], F32, kind="ExternalOutput").ap()

    wq_r = wq_d.rearrange("(k p) n -> p k n", p=128)
    wkv_r = wkv_d.rearrange("(k p) n -> p k n", p=128)
    wo_r = wo_d.rearrange("(k p) n -> p k n", p=128)
    wff1_r = wff1_d.rearrange("(k p) n -> p k n", p=128)
    wff2_r = wff2_d.rearrange("(f p) n -> p f n", p=128)

    with tile.TileContext(nc) as tc:
        _build_tile(tc, nc, x_d, ctx_d, wq_r, wkv_r, wo_r, wff1_r, wff2_r, y_d,
                    ident8_d, e2_d)
    _split_excess_waits(nc)
    return nc


def _build_tile(tc, nc, x_d, ctx_d, wq_r, wkv_r, wo_r, wff1_r, wff2_r, y_d,
                ident8_d, e2_d):
    from contextlib import ExitStack

    ctx = ExitStack()
    with ctx:
        constp = ctx.enter_context(tc.tile_pool(name="const", bufs=1))
        pers = ctx.enter_context(tc.tile_pool(name="pers", bufs=1))

        eps_t = constp.tile([128, 1], F32)
        nc.vector.memset(eps_t, EPS)
        expb_t = constp.tile([128, 1], F32)
        nc.vector.memset(expb_t, EXP_BIAS)
        # all-ones row for broadcasting softmax sums across partitions
        e2 = constp.tile([33, 128], F32R)
        nc.sync.dma_start(out=e2, in_=e2_d)

        # persistent activations
        xnT = pers.tile([128, KC, TPC], BF16)        # LN(x)^T       6KB
        xnT8 = pers.tile([128, KC, TPC], FP8)        # fp8 view for Q 3KB
        cnT = pers.tile([128, KC, J], BF16)          # LN(ctx)^T    24KB
        qT8 = pers.tile([128, KC + 1, TPC], FP8)     # q^T pairs   3.5KB
        kTz_lo = pers.tile([128, JT, 2, 128], FP8)   # k + zero sub 4KB
        kTz_hi = pers.tile([128, JT, 2, 128], FP8)   # same @ parts 64:128
        v8 = pers.tile([128, J], BF16)               # v^T rows 64:128  4KB
        v_tok = pers.tile([128, JT, DH], BF16)       # v token-major 2KB
        vaug = pers.tile([128, JT, 128], FP8)        # v | 2.0 | pad 2KB
        outT8 = pers.tile([128, KC, TPC], FP8)       # attn out^T   3KB
        hT = pers.tile([128, FH, TPC], BF16)         # swiglu hidden 24KB

        nc.vector.memset(kTz_lo, 0.0)
        nc.gpsimd.memset(kTz_hi, 0.0)
        nc.vector.memset(vaug, 0.0)
        nc.vector.memset(vaug[:, :, DH : DH + 1], ONES_COL)
        # pad slot: the sim DR rhs reads slot hp+1 against zero weights; it
        # must hold finite values (0 * inf/NaN would poison the PSUM)
        nc.gpsimd.memset(qT8[:, KC, :], 0.0)

        # resident weights
        wq_sb = pers.tile([128, KC, DIM], FP8)       # 4.5KB
        nc.sync.dma_start(out=wq_sb, in_=wq_r)
        wkv_sb = pers.tile([128, KC, 2 * DH], BF16)  # 1.5KB
        nc.sync.dma_start(out=wkv_sb, in_=wkv_r)
        wo_sb = pers.tile([128, KC, DIM], FP8)       # 4.5KB
        nc.sync.dma_start(out=wo_sb, in_=wo_r)
        w2_sb = pers.tile([128, FH, DIM], BF16)      # 36KB (streamed in FF1)
        ident8 = constp.tile([128, 128], FP8)
        nc.sync.dma_start(out=ident8, in_=ident8_d)

        # FF1 pools live for the whole build: chunks are emitted early
        # (during ctx LN) and inside the attention loop
        ffps = ctx.enter_context(tc.tile_pool(name="ffps", bufs=1, space="PSUM"))
        w1p = ctx.enter_context(tc.tile_pool(name="wff1", bufs=4))
        silp = ctx.enter_context(tc.tile_pool(name="sil", bufs=3))
        apool = ctx.enter_context(tc.tile_pool(name="attn", bufs=10))
        rbp = ctx.enter_context(tc.tile_pool(name="rb", bufs=3))

        def sim_exp(h, jp, ps_pool, ps_tag):
            par = h & 1
            hp = h >> 1
            kTz = kTz_hi if par else kTz_lo
            lo = DH * par
            ps = ps_pool.tile([128, 2, TPC], F32, tag=ps_tag)
            for u in range(2):
                jt = 2 * jp + u
                nc.tensor.matmul(
                    ps[:, u, :],
                    lhsT=kTz[lo : lo + DH, jt, :, :],
                    rhs=qT8[lo : lo + DH, hp : hp + 2, :],
                    start=True,
                    stop=True,
                    perf_mode=DR,
                )
            p8 = apool.tile([128, 2, TPC], FP8, tag="p")
            ps_f = ps.rearrange("p a b -> p (a b)")
            if EXP_ENGINES[jp] == "a":
                nc.scalar.activation(
                    out=p8.bitcast(U8).rearrange("p a b -> p (a b)"),
                    in_=ps_f,
                    func=AF.Copy,
                    scale=SCH_A,
                    bias=SCH_B,
                )
            else:
                nc.vector.tensor_scalar(
                    out=p8.bitcast(U8).rearrange("p a b -> p (a b)"),
                    in0=ps_f,
                    scalar1=SCH_A,
                    scalar2=SCH_B,
                    op0=MULT,
                    op1=ADD,
                )
            return p8

        def ff_chunk(f):
            nc.sync.dma_start(out=w2_sb[:, f, :], in_=wff2_r[:, f, :])
            wg = w1p.tile([128, KC, 128], BF16, tag="wg")
            nc.sync.dma_start(
                out=wg, in_=wff1_r[:, :, FFI + f * 128 : FFI + (f + 1) * 128]
            )
            pg = ffps.tile([128, TPC], F32, tag="ff", name=f"pg{f}")
            for k in range(KC):
                nc.tensor.matmul(
                    pg,
                    lhsT=wg[:, k, :],
                    rhs=xnT[:, k, :],
                    start=(k == 0),
                    stop=(k == KC - 1),
                )
            sil = silp.tile([128, TPC], BF16, tag="sig")
            nc.scalar.activation(out=sil, in_=pg, func=AF.Silu)
            wu = w1p.tile([128, KC, 128], BF16, tag="wu")
            nc.sync.dma_start(out=wu, in_=wff1_r[:, :, f * 128 : (f + 1) * 128])
            pu = ffps.tile([128, TPC], F32, tag="ff", name=f"pu{f}")
            for k in range(KC):
                nc.tensor.matmul(
                    pu,
                    lhsT=wu[:, k, :],
                    rhs=xnT[:, k, :],
                    start=(k == 0),
                    stop=(k == KC - 1),
                )
            nc.vector.tensor_mul(out=hT[:, f, :], in0=pu, in1=sil)

        # ---------- phase A+B: LN -> DMA-transpose into xnT / cnT --------
        # stats for all tiles of a group are batched so the expensive
        # sqrt/reciprocal run once per group instead of once per tile
        with (
            tc.tile_pool(name="io", bufs=9) as iop,
            tc.tile_pool(name="stats", bufs=2) as stp,
        ):
            def ln_group(dram, dst, t0, nm):
                # 4 tiles per group: one sqrt + one reciprocal per group
                xts = []
                mvg = stp.tile([128, 4, nc.vector.BN_AGGR_DIM], F32, tag="mv")
                for u in range(4):
                    t = t0 + u
                    xt = iop.tile([128, DIM], BF16, tag="ln_in", name=f"ln{nm}_{t}")
                    nc.sync.dma_start(out=xt, in_=dram[t * 128 : (t + 1) * 128, :])
                    xts.append(xt)
                    xg = xt.rearrange("p (s d) -> p s d", d=256)
                    st = stp.tile(
                        [128, 3, nc.vector.BN_STATS_DIM], F32, tag="bn"
                    )
                    for sg in range(3):
                        nc.vector.bn_stats(out=st[:, sg, :], in_=xg[:, sg, :])
                    nc.vector.bn_aggr(out=mvg[:, u, :], in_=st)
                stdg = stp.tile([128, 4], F32, tag="sd")
                nc.scalar.activation(
                    out=stdg, in_=mvg[:, :, 1], func=AF.Sqrt, bias=eps_t, scale=1.0
                )
                rsg = stp.tile([128, 4], F32, tag="rs")
                nc.vector.reciprocal(out=rsg, in_=stdg)
                for u in range(4):
                    t = t0 + u
                    xn = iop.tile([128, DIM], BF16, tag="ln_out")
                    nc.vector.tensor_scalar(
                        out=xn, in0=xts[u], scalar1=mvg[:, u, 0:1],
                        scalar2=rsg[:, u : u + 1], op0=SUB, op1=MULT
                    )
                    nc.sync.dma_start_transpose(
                        out=dst[:, :, t * 128 : (t + 1) * 128], in_=xn
                    )

            ln_group(x_d, xnT, 0, "x")
            for kp in range(KC // 2):
                nc.scalar.copy(
                    out=xnT8[:, 2 * kp : 2 * kp + 2, :].rearrange("p a b -> p (a b)"),
                    in_=xnT[:, 2 * kp : 2 * kp + 2, :].rearrange("p a b -> p (a b)"),
                )

            # ---------- phase C: qT8 = (Wq^T @ xn^T) fp8 DoubleRow -------
            with tc.tile_pool(name="mmp", bufs=2, space="PSUM") as mmp:
                for i in range(KC):
                    ps = mmp.tile([128, TPC], F32, tag="mm")
                    for kp in range(KC // 2):
                        nc.tensor.matmul(
                            ps,
                            lhsT=wq_sb[:, 2 * kp : 2 * kp + 2, i * 128 : (i + 1) * 128],
                            rhs=xnT8[:, 2 * kp : 2 * kp + 2, :],
                            start=(kp == 0),
                            stop=(kp == KC // 2 - 1),
                            perf_mode=DR,
                        )
                    nc.scalar.copy(out=qT8[:, i, :], in_=ps)

                ff_chunk(0)
                ff_chunk(1)
                for g in range(JT // 4):
                    ln_group(ctx_d, cnT, 4 * g, "c")

                # ------ phase D: kv chunks -> kTz_lo, v8 ------------------
                h0_p8s = []
                for c in range(J // TPC):
                    ps = mmp.tile([128, TPC], F32, tag="mm")
                    for k in range(KC):
                        nc.tensor.matmul(
                            ps,
                            lhsT=wkv_sb[:, k, :],
                            rhs=cnT[:, k, c * TPC : (c + 1) * TPC],
                            start=(k == 0),
                            stop=(k == KC - 1),
                        )
                    # k rows 0:64 -> kTz_lo sub 0 of j-tiles 4c..4c+4
                    nc.scalar.copy(
                        out=kTz_lo[0:DH, 4 * c : 4 * c + 4, 0, :],
                        in_=ps[0:DH, :].rearrange("p (a b) -> p a b", b=128),
                    )
                    nc.scalar.copy(
                        out=v8[DH:128, c * TPC : (c + 1) * TPC], in_=ps[DH:128, :]
                    )
                    ff_chunk(2 + c)
                    # head 0's sims for this chunk's j-tiles overlap ctx LN
                    for jp in (2 * c, 2 * c + 1):
                        h0_p8s.append(sim_exp(0, jp, mmp, "s0"))

                # replicate k into partitions 64:128 (SBUF->SBUF DMA crosses
                # partitions freely)
                nc.sync.dma_start(
                    out=kTz_hi[DH:128, :, 0, :], in_=kTz_lo[0:DH, :, 0, :]
                )
                # v to token-major via DMA transpose, then cast to fp8
                nc.sync.dma_start_transpose(out=v_tok, in_=v8[DH:128, :])
                nc.scalar.copy(out=vaug[:, :, 0:DH], in_=v_tok)

        # ---------- phase F: attention, FF1 interleaved two f-chunks per
        # head so the tensor engine never starves while ACT/DVE run exp ----
        with (
            tc.tile_pool(name="spsum", bufs=2, space="PSUM") as spsum,
            tc.tile_pool(name="opsum", bufs=1, space="PSUM") as opsum,
            tc.tile_pool(name="rbps", bufs=1, space="PSUM") as rbps,
        ):
            po_pair = {}
            sums2 = None
            ff_iter = iter(range(6, FH))
            pending = []  # deferred pair-finish closures

            def finish_pair(hp, sums2_, po_e, po_o):
                recips = {}

                def run_early():
                    if "r" in recips:
                        return
                    recip2 = rbp.tile([33, TPC], F32R, tag="rec")
                    with nc.allow_low_precision(reason="f32r == f32 bits"):
                        nc.vector.reciprocal(out=recip2, in_=sums2_)
                    recips["r"] = recip2

                def run():
                    recip2 = recips["r"]
                    rb = rbp.tile([128, 2 * TPC], F32, tag="rbb", name=f"rb{hp}")
                    for p2 in range(2):
                        rbq = rbps.tile([128, TPC], F32, tag="rbq")
                        nc.tensor.matmul(
                            rbq,
                            lhsT=e2[32 * p2 : 32 * p2 + 1, :],
                            rhs=recip2[32 * p2 : 32 * p2 + 1, :],
                            start=True,
                            stop=True,
                        )
                        nc.scalar.copy(
                            out=rb[:, p2 * TPC : (p2 + 1) * TPC], in_=rbq
                        )
                    for p2, po_ in ((0, po_e), (1, po_o)):
                        nc.vector.tensor_mul(
                            out=outT8[p2 * DH : (p2 + 1) * DH, hp, :],
                            in0=po_[0:DH, :],
                            in1=rb[p2 * DH : (p2 + 1) * DH, p2 * TPC : (p2 + 1) * TPC],
                        )
                return run_early, run

            for h in range(HEADS):
                par = h & 1
                hp = h >> 1
                kTz = kTz_hi if par else kTz_lo
                lo = DH * par
                for early, _ in pending:
                    early()
                    break
                if par == 0:
                    sums2 = rbp.tile([33, TPC], F32R, tag="sums", name=f"sums{hp}")
                # sims + exp first, ff matmuls woven in to cover exp latency
                if h == 0:
                    p8s = h0_p8s
                    for _ in range(2):
                        fi = next(ff_iter, None)
                        if fi is not None:
                            ff_chunk(fi)
                p8s = p8s if h == 0 else []
                for jp in range(0 if h else 0, JT // 2):
                    if h == 0:
                        break
                    ps = spsum.tile([128, 2, TPC], F32, tag="s")
                    for u in range(2):
                        jt = 2 * jp + u
                        nc.tensor.matmul(
                            ps[:, u, :],
                            lhsT=kTz[lo : lo + DH, jt, :, :],
                            rhs=qT8[lo : lo + DH, hp : hp + 2, :],
                            start=True,
                            stop=True,
                            perf_mode=DR,
                        )
                    p8 = apool.tile([128, 2, TPC], FP8, tag="p")
                    p8s.append(p8)
                    eng = EXP_ENGINES[jp]
                    ps_f = ps.rearrange("p a b -> p (a b)")
                    if eng == "a":
                        nc.scalar.activation(
                            out=p8.bitcast(U8).rearrange("p a b -> p (a b)"),
                            in_=ps_f,
                            func=AF.Copy,
                            scale=SCH_A,
                            bias=SCH_B,
                        )
                    else:
                        engine = nc.vector if eng == "d" else nc.gpsimd
                        engine.tensor_scalar(
                            out=p8.bitcast(U8).rearrange("p a b -> p (a b)"),
                            in0=ps_f,
                            scalar1=SCH_A,
                            scalar2=SCH_B,
                            op0=MULT,
                            op1=ADD,
                        )
                    if jp == 1 or (jp == 3 and par == 0):
                        fi = next(ff_iter, None)
                        if fi is not None:
                            ff_chunk(fi)
                    if jp == 3:
                        # previous pair finishes while sims continue
                        for _, run in pending:
                            run()
                        pending.clear()
                po = opsum.tile([128, TPC], F32, tag=f"po{par}", name=f"po_{h}")
                for jp in range(JT // 2):
                    nc.tensor.matmul(
                        po,
                        lhsT=vaug[:, 2 * jp : 2 * jp + 2, :],
                        rhs=p8s[jp],
                        start=(jp == 0),
                        stop=(jp == JT // 2 - 1),
                        perf_mode=DR,
                    )
                nc.scalar.copy(
                    out=sums2[32 * par : 32 * par + 1, :],
                    in_=po[DH : DH + 1, :],
                )
                po_pair[par] = po
                if par == 1:
                    pending.append(
                        finish_pair(hp, sums2, po_pair[0], po_pair[1])
                    )
            for early, run in pending:
                early()
                run()
            pending.clear()

        # ---------- phase G: y = outT8.Wo (fp8 DR) + hT.Wff2 (bf16) ------
        NCH = ((0, 512), (512, 256))
        with (
            tc.tile_pool(name="gps", bufs=2, space="PSUM") as gps,
            tc.tile_pool(name="yout", bufs=2) as yp,
        ):
            for t in range(TT):
                pg = gps.tile([128, DIM], F32, tag="g")
                for n0, nw in NCH:
                    for kp in range(KC // 2):
                        nc.tensor.matmul(
                            pg[:, n0 : n0 + nw],
                            lhsT=outT8[:, 2 * kp : 2 * kp + 2, t * 128 : (t + 1) * 128],
                            rhs=wo_sb[:, 2 * kp : 2 * kp + 2, n0 : n0 + nw],
                            start=(kp == 0),
                            stop=False,
                            perf_mode=DR,
                        )
                    for f in range(FH):
                        nc.tensor.matmul(
                            pg[:, n0 : n0 + nw],
                            lhsT=hT[:, f, t * 128 : (t + 1) * 128],
                            rhs=w2_sb[:, f, n0 : n0 + nw],
                            start=False,
                            stop=(f == FH - 1),
                        )
                ysb = yp.tile([128, DIM], F32, tag="y")
                nc.scalar.activation(out=ysb, in_=pg, func=AF.Copy, scale=Y_SCALE)
                nc.sync.dma_start(out=y_d[t * 128 : (t + 1) * 128, :], in_=ysb)


@functools.lru_cache(maxsize=1)
def _build():
    _install_drain_patch()
    nc = bass.Bass("TRN2", target_bir_lowering=False, debug=False, num_devices=NCORES)
    return _emit(nc)


# ------------------------------------------------------ ntff hook shim
def _ensure_ntff_hook():
    """This image's `antenv` lacks `axon_hooks`; synthesize it so
    run_bass_kernel_spmd(trace=True) can capture NTFF profiles via the
    libaxon_pjrt C ABI (same recipe as trn_boot._ntff_profile_via_ctypes)."""
    import contextlib
    import ctypes
    import os
    import sys
    import types

    try:
        from antenv.axon_hooks import get_axon_ntff_profile_hook  # noqa: F401

        return
    except ImportError:
        pass
    import antenv

    mod = types.ModuleType("antenv.axon_hooks")
    holder = {"hook": None}
    mod.set_axon_ntff_profile_hook = lambda h: holder.__setitem__("hook", h)
    mod.get_axon_ntff_profile_hook = lambda: holder["hook"]
    sys.modules["antenv.axon_hooks"] = mod
    antenv.axon_hooks = mod

    so_path = "/opt/axon/libaxon_pjrt.so"
    if not os.path.exists(so_path):
        return
    lib = ctypes.CDLL(so_path)
    if not hasattr(lib, "axon_start_nrt_profile"):
        return
    lib.axon_start_nrt_profile.argtypes = [
        ctypes.POINTER(ctypes.c_int64),
        ctypes.c_size_t,
    ]
    lib.axon_start_nrt_profile.restype = ctypes.c_int64
    lib.axon_stop_nrt_profile.argtypes = [ctypes.c_char_p]
    lib.axon_stop_nrt_profile.restype = ctypes.c_int64

    @contextlib.contextmanager
    def _hook(output_dir, device_ids):
        import jax

        jax.devices()
        if device_ids:
            ids = (ctypes.c_int64 * len(device_ids))(*device_ids)
            rc = lib.axon_start_nrt_profile(ids, len(device_ids))
        else:
            rc = lib.axon_start_nrt_profile(None, 0)
        if rc != 0:
            raise RuntimeError(f"axon_start_nrt_profile rc={rc}")
        try:
            yield
        finally:
            n = lib.axon_stop_nrt_profile(str(output_dir).encode())
            print(f"ntff profile: {n} file(s) written to {output_dir}")

    mod.set_axon_ntff_profile_hook(_hook)


# ---------------------------------------------------------------- entry
TRACE = False  # test harnesses can flip this to capture an NTFF profile
LAST = None
IDENT8 = np.eye(128, dtype=np.float32).astype(ml_dtypes.float8_e4m3)
E2 = np.zeros((33, 128), np.float32)
E2[0, :] = 1.0
E2[32, :] = 1.0


def _bf16(a):
    return np.ascontiguousarray(np.asarray(a, np.float32)).astype(ml_dtypes.bfloat16)


def _fp8(a):
    return np.ascontiguousarray(np.asarray(a, np.float32)).astype(
        ml_dtypes.float8_e4m3
    )


def kernel(**inputs):
    x = np.asarray(inputs["x"], dtype=np.float32)
    context = np.asarray(inputs["context"], dtype=np.float32)
    gx = np.asarray(inputs["gamma_x"], dtype=np.float32)
    gc = np.asarray(inputs["gamma_c"], dtype=np.float32)
    # fold LN gammas, the attention scale and fp8 range scales into weights
    wq = _fp8(gx[:, None] * np.asarray(inputs["Wq"]) * (DH**-0.5) * 2.0**7)
    wkv = _bf16(gc[:, None] * np.asarray(inputs["Wkv"]) * 2.0**4)
    wo = _fp8(np.asarray(inputs["Wo"]) * 2.0**5)
    wff1 = _bf16(gx[:, None] * np.asarray(inputs["Wff1"]))
    wff2 = _bf16(np.asarray(inputs["Wff2"]) * 2.0**8)
    xb = _bf16(x)
    cb = _bf16(context)

    in_maps = []
    for c in range(NCORES):
        b, t0 = c // 2, (c % 2) * TPC
        in_maps.append(
            {
                "x": np.ascontiguousarray(xb[b, t0 : t0 + TPC]),
                "ctx": cb[b],
                "wq": wq,
                "wkv": wkv,
                "wo": wo,
                "wff1": wff1,
                "wff2": wff2,
                "ident8": IDENT8,
                "e2": E2,
            }
        )

    nc = _build()
    if TRACE:
        _ensure_ntff_hook()
    res = run_bass_kernel_spmd(nc, in_maps, list(range(NCORES)), trace=TRACE)
    global LAST
    LAST = res
    out = np.empty((B, N, DIM), np.float32)
    for c in range(NCORES):
        b, t0 = c // 2, (c % 2) * TPC
        out[b, t0 : t0 + TPC] = res.results[c]["y"]
    return out
